# revision 38
# baseline (speedup 1.0000x reference)
"""Multi-head causal attention on 8 Trainium2 NeuronCores.

nn_MultiHeadAttention_37933151158277: x[2,2048,2048] f32, causal mask,
W_qkv[6144,2048], W_o[2048,2048]. Tensor-parallel over heads (2 per
core): qkv_proj output and W_o input split along the head dimension,
x replicated. Host sums the 8 partial y outputs (the TP unshard).

Per core:
  phase 1 - QKV projection in bf16. Host supplies x^T and the weight
      slices pre-transposed AND pre-tiled partition-major, so every
      load is ONE dma_start with 16KB-contiguous rows (HWDGE DMAs are
      FIFO-serialized per ring and descriptor-bound below 64KB - many
      small tile DMAs were costing a 34us startup bubble). Weights ride
      the Activation-engine HWDGE ring, the xt chunk stream and y
      output ride the SP ring, so both transfer in parallel. Q^T/K^T
      land as [d_k=128, tok] f32r and V as [tok, d_k] f32r via PSUM-f32
      accumulation over 16 d-tiles.
  phase 2 - attention per (batch, head), two streams interleaved
      entry-by-entry, PV/denominator two-to-three entries behind the
      score matmuls (shallow during each block's ramp-in, deep in
      steady state) so exp latency never blocks the in-order PE queue.
      Scores computed transposed: S^T[k,q] = K^T_tile.T @ Q^T; exp on
      scalar engine (the only exp-capable engine - kept free of copies
      in this phase); P @ V consumes exp(S^T) with V stationary. The
      softmax denominator is accumulated on the PE at entry-PAIR
      granularity: one DVE add merges two exp tiles, then a single
      accumulating [1, q] ones-vector matmul per pair rides alongside
      the PV chain (halves the denominator's PE rows). The normalization tail is PE-free: reciprocal_approx_fast
      (DVE) -> partition_broadcast (GpSimd) -> tensor_mul (DVE), so the
      PE rolls straight into the next q-block. No max-subtraction
      (scores are O(1) by construction). Mask handled block-wise at
      trace time: fully-masked blocks skipped, fully-valid blocks skip
      the mask multiply, mixed blocks get a content-deduped mask-tile
      multiply (tiles pre-packed by the host, one DMA).
  phase 3 - partial out-projection y_c = attn_out @ W_o[:, cols]^T in
      f32r, emitted as PE filler one q-block behind attention (pairs of
      tiles chunk-interleaved so PSUM-copy latency never serializes the
      matmul stream). Each [128, 2048] y tile is copied from PSUM on
      vector (3/4) + scalar (1/4) and leaves as ONE bf16 DMA.

Host: y = sum_c y_c in f32.

Measured: ~371.4 us on hardware per core (8 cores SPMD), rel err 3.0e-3
(scale-relative absmax; the error is dominated by the bf16 QKV inputs
and bf16 y output, both well inside the 2e-2 gate). Baseline was 474 us.
"""
import sys
if '/opt/trn_rl_repo' not in sys.path:
    sys.path.insert(0, '/opt/trn_rl_repo')

import numpy as np

B, S, D = 2, 2048, 2048
H, DK = 16, 128
NCORES = 8
HPC = H // NCORES            # heads per core
T = B * S                    # tokens
QB = 512                     # q-block width (free dim of S^T / PV matmuls)
NKT = S // 128               # k tiles per batch (16)
NQB = S // QB                # q blocks per batch (4)
NCH = T // QB                # token chunks (8)
NDT = D // 128               # d_model tiles (16)

_cache = {}


def _analyze_mask(m2):
    """m2: [S, S] bool, m2[q, k]. Returns blocks[qb] = list of entries
    (j, q0c, mm0, mm1) ascending j:
      q0c: first q col (within block) to compute, mm0..mm1: mask-mul range
      (None if block fully valid over [q0c, QB)).
    """
    blocks = []
    for qb in range(NQB):
        entries = []
        for j in range(NKT):
            blk = m2[qb * QB:(qb + 1) * QB, j * 128:(j + 1) * 128]
            col_any = blk.any(axis=1)
            if not col_any.any():
                continue
            col_all = blk.all(axis=1)
            q0 = int(np.argmax(col_any))
            # q1: start of the trailing fully-valid run
            rev = col_all[::-1]
            run = int(np.argmin(rev)) if not rev.all() else QB
            q1 = QB - run
            if q1 <= q0:
                entries.append((j, q0, None, None))
            else:
                entries.append((j, q0, q0, q1))
        if entries:
            qmin = min(e[1] for e in entries)
            j, q0, m0, m1 = entries[0]
            if q0 > qmin:
                # first entry must cover every column later entries write
                entries[0] = (j, qmin, qmin, m1 if m1 is not None else q0)
        blocks.append(entries)
    return blocks


def _mask_tile_specs(m2, blocks):
    """Deterministic discovery of unique (content-deduped) mask tiles.
    Returns (specs, key2idx): specs[i] = (qb, j, m0, m1)."""
    specs = []
    key2idx = {}
    for qb in range(NQB):
        for (j, q0c, m0, m1) in blocks[qb]:
            if m0 is None:
                continue
            key = m2[qb * QB + m0:qb * QB + m1,
                     j * 128:(j + 1) * 128].tobytes()
            if key not in key2idx:
                key2idx[key] = len(specs)
                specs.append((qb, j, m0, m1))
    return specs, key2idx


def _build(mask_bool):
    from collections import deque
    from contextlib import ExitStack
    import concourse.tile as tile
    from concourse import bacc, mybir

    f32 = mybir.dt.float32
    f32r = mybir.dt.float32r
    bf16 = mybir.dt.bfloat16
    EXP = mybir.ActivationFunctionType.Exp
    scale = 1.0 / np.sqrt(DK)

    m2 = mask_bool
    blocks = _analyze_mask(m2)
    mspecs, mkey2idx = _mask_tile_specs(m2, blocks)
    ntiles = max(1, len(mspecs))

    nc = bacc.Bacc("TRN2", target_bir_lowering=False, debug=False)
    xt_d = nc.dram_tensor("xt", [NCH, 128, NDT * 512], bf16,
                          kind="ExternalInput")
    wqk_d = nc.dram_tensor("wqk", [128, NDT * 512], bf16,
                           kind="ExternalInput")
    wv_d = nc.dram_tensor("wv", [128, NDT * 256], bf16,
                          kind="ExternalInput")
    wo_d = nc.dram_tensor("wo", [128, HPC * D], f32r, kind="ExternalInput")
    mt_d = nc.dram_tensor("mt", [128, ntiles * QB], f32r,
                          kind="ExternalInput")
    y_d = nc.dram_tensor("y", [T, D], bf16, kind="ExternalOutput")

    with tile.TileContext(nc) as tc:
        with ExitStack() as stack:
            stack.enter_context(
                nc.allow_low_precision(reason="float32r/bf16 matmul inputs"))
            qkt_pool = stack.enter_context(tc.tile_pool(name="qkt", bufs=1))
            v_pool = stack.enter_context(tc.tile_pool(name="vsb", bufs=1))
            att_pool = stack.enter_context(tc.tile_pool(name="att", bufs=1))
            cst_pool = stack.enter_context(tc.tile_pool(name="cst", bufs=1))
            wo_pool = stack.enter_context(tc.tile_pool(name="wo", bufs=1))
            msk_pool = stack.enter_context(tc.tile_pool(name="msk", bufs=1))

            # persistent SBUF
            qt_sb = [qkt_pool.tile([128, T], f32r, tag=f"qt{h}", name=f"qt{h}")
                     for h in range(HPC)]
            kt_sb = [qkt_pool.tile([128, T], f32r, tag=f"kt{h}", name=f"kt{h}")
                     for h in range(HPC)]
            v_sb = v_pool.tile([128, (T // 128) * 256], f32r, tag="v")
            at_sb = [att_pool.tile([128, T], f32r, tag=f"at{h}", name=f"at{h}")
                     for h in range(HPC)]

            # single-DMA weight/mask loads on the ACT HWDGE ring (the SP
            # ring is left for the xt chunk stream + y output), issued
            # before any compute-engine init so the ring starts instantly
            wqk_sb = wo_pool.tile([128, NDT * 512], bf16, tag="wqk")
            qw = NDT * 128
            for qi in range(4):
                nc.scalar.dma_start(wqk_sb[:, qi * qw:(qi + 1) * qw],
                                    wqk_d.ap()[:, qi * qw:(qi + 1) * qw])
            wv_sb = wo_pool.tile([128, NDT * 256], bf16, tag="wv")
            nc.scalar.dma_start(wv_sb[:], wv_d.ap()[:, :])
            wo_sb = wo_pool.tile([128, HPC * D], f32r, tag="wo")
            nc.scalar.dma_start(wo_sb[:], wo_d.ap()[:, :])
            msk_sb = msk_pool.tile([128, ntiles * QB], f32r, tag="msk")
            nc.scalar.dma_start(msk_sb[:], mt_d.ap()[:, :])

            ones_f = cst_pool.tile([128, 128], f32, tag="ones_f")
            nc.vector.memset(ones_f[:], 1.0)
            ones_col = cst_pool.tile([128, 1], f32r, tag="ones_c")
            nc.scalar.copy(ones_col[:], ones_f[:, 0:1])

            def mask_tile(j, qb, m0, m1):
                key = m2[qb * QB + m0:qb * QB + m1,
                         j * 128:(j + 1) * 128].tobytes()
                i = mkey2idx[key]
                return msk_sb[:, i * QB:i * QB + (m1 - m0)]

            # ---------------- phase 1: QKV projection (bf16) ----------------
            with ExitStack() as p1:
                xt_pool = p1.enter_context(tc.tile_pool(name="xt", bufs=2))
                qk_ps_pool = p1.enter_context(
                    tc.tile_pool(name="ps_qk", bufs=4, space="PSUM"))
                v_ps_pool = p1.enter_context(
                    tc.tile_pool(name="ps_v", bufs=4, space="PSUM"))

                for c in range(NCH):
                    xt_t = xt_pool.tile([128, NDT * 512], bf16, tag="xt")
                    if c == 0:
                        xq = NDT * 128
                        for qi in range(4):
                            nc.sync.dma_start(
                                xt_t[:, qi * xq:(qi + 1) * xq],
                                xt_d.ap()[0, :, qi * xq:(qi + 1) * xq])
                    else:
                        nc.sync.dma_start(xt_t[:], xt_d.ap()[c])
                    qk_ps = [qk_ps_pool.tile([128, 512], f32, tag="qk",
                                             name="qkps")
                             for _ in range(4)]
                    v_ps = [v_ps_pool.tile([128, 256], f32, tag="v",
                                           name="vps")[:]
                            for _ in range(4)]

                    def emit_qk(kd):
                        st, sp = kd == 0, kd == NDT - 1
                        for e in range(4):
                            nc.tensor.matmul(
                                qk_ps[e][:],
                                wqk_sb[:, kd * 512 + e * 128:
                                       kd * 512 + (e + 1) * 128],
                                xt_t[:, kd * 512:(kd + 1) * 512],
                                start=st, stop=sp)

                    def emit_v(kd):
                        st, sp = kd == 0, kd == NDT - 1
                        for tl in range(4):
                            nc.tensor.matmul(
                                v_ps[tl],
                                xt_t[:, kd * 512 + tl * 128:
                                     kd * 512 + tl * 128 + 128],
                                wv_sb[:, kd * 256:(kd + 1) * 256],
                                start=st, stop=sp)

                    if c == 0:
                        # wv lands after wqk on the ACT ring: run all qk
                        # matmuls first so the PE isn't blocked on wv
                        for kd in range(NDT):
                            emit_qk(kd)
                        for kd in range(NDT):
                            emit_v(kd)
                    else:
                        for kd in range(NDT):
                            emit_qk(kd)
                            emit_v(kd)

                    dsts = [qt_sb[0], qt_sb[1], kt_sb[0], kt_sb[1]]
                    for tl in range(4):
                        tok = c * 4 + tl
                        nc.scalar.copy(
                            v_sb[:, tok * 256:(tok + 1) * 256], v_ps[tl])
                    for e in range(4):
                        nc.vector.tensor_copy(
                            dsts[e][:, c * 512:(c + 1) * 512], qk_ps[e][:])

            # ---------- phase 2 + 3: attention + projection ----------
            with ExitStack() as p2:
                e_pool = p2.enter_context(tc.tile_pool(name="e", bufs=8))
                rcp_pool = p2.enter_context(tc.tile_pool(name="rcp", bufs=2))
                b_pool = p2.enter_context(tc.tile_pool(name="bsb", bufs=2))
                ysb_pool = p2.enter_context(tc.tile_pool(name="ysb", bufs=3))
                o_ps_pool = p2.enter_context(
                    tc.tile_pool(name="ps_o", bufs=2, space="PSUM"))
                d_ps_pool = p2.enter_context(
                    tc.tile_pool(name="ps_d", bufs=2, space="PSUM"))
                y_ps_pool = p2.enter_context(
                    tc.tile_pool(name="ps_y", bufs=2, space="PSUM"))
                s_ps_pool = p2.enter_context(
                    tc.tile_pool(name="ps_s", bufs=2, space="PSUM"))

                class QbStream:
                    """One (batch, head) stream over a q-block. One entry
                    (k-tile) per step; PV + denominator run two entries
                    behind the S matmul so exp latency stays off the
                    in-order PE queue."""

                    def __init__(self, b, h, qb):
                        self.b, self.h, self.qb = b, h, qb
                        self.tb = b * S
                        self.entries = blocks[qb]
                        self.ne = len(self.entries)
                        self.o_ps = o_ps_pool.tile([128, QB], f32, tag="o",
                                                   name="ops")
                        self.d_ps = d_ps_pool.tile([1, QB], f32, tag="d",
                                                   name="dps")
                        self.qcol = self.tb + qb * QB
                        self.pends = deque()
                        self.elist = {}
                        self.gi = 0

                    def s_and_exp(self, ent):
                        j, q0c, m0, m1 = ent
                        s_ps = s_ps_pool.tile([128, QB], f32, tag="s",
                                              name="sps")
                        nc.tensor.matmul(
                            s_ps[:, q0c:QB],
                            kt_sb[self.h][:, self.tb + j * 128:
                                          self.tb + (j + 1) * 128],
                            qt_sb[self.h][:, self.qcol + q0c:self.qcol + QB],
                            start=True, stop=True)
                        e_sb = e_pool.tile([128, QB], f32r, tag="e",
                                           name="esb")
                        nc.scalar.activation(e_sb[:, q0c:QB], s_ps[:, q0c:QB],
                                             EXP, scale=scale)
                        if m0 is not None:
                            nc.vector.tensor_mul(
                                e_sb[:, m0:m1], e_sb[:, m0:m1],
                                mask_tile(j, self.qb, m0, m1))
                        return e_sb

                    def pv_and_d(self, ent, gi, e_sb):
                        j, q0c, m0, m1 = ent
                        st, sp = gi == 0, gi == self.ne - 1
                        nc.tensor.matmul(
                            self.o_ps[:, q0c:QB],
                            v_sb[:, (self.b * NKT + j) * 256 + self.h * 128:
                                 (self.b * NKT + j) * 256 + (self.h + 1) * 128],
                            e_sb[:, q0c:QB],
                            start=st, stop=sp)
                        # denominator at pair granularity: sum two exp
                        # tiles on DVE, one accumulating [1, q] matmul
                        self.elist[gi] = (ent, e_sb)
                        if gi % 2 == 1:
                            (_, c0, _, _), e0 = self.elist[gi - 1]
                            (_, c1, _, _), e1 = self.elist[gi]
                            nc.vector.tensor_add(
                                e0[:, c1:QB], e0[:, c1:QB], e1[:, c1:QB])
                            nc.tensor.matmul(
                                self.d_ps[0:1, c0:QB],
                                ones_col[:],
                                e0[:, c0:QB],
                                start=(gi == 1), stop=sp)
                        elif sp:
                            nc.tensor.matmul(
                                self.d_ps[0:1, q0c:QB],
                                ones_col[:],
                                e_sb[:, q0c:QB],
                                start=(gi == 0), stop=True)

                    def step(self):
                        if self.gi < self.ne:
                            ent = self.entries[self.gi]
                            e_sb = self.s_and_exp(ent)
                            depth = 2 if self.gi < 4 else 3
                            if len(self.pends) >= depth:
                                self.pv_and_d(*self.pends.popleft())
                            self.pends.append((ent, self.gi, e_sb))
                            self.gi += 1
                            return True
                        return False

                    def finish_pv(self):
                        while self.pends:
                            self.pv_and_d(*self.pends.popleft())

                def emit_tail(streams):
                    # PE-free normalization: 1/d on DVE, broadcast on
                    # GpSimd, o * (1/d) on DVE straight out of PSUM.
                    for st in streams:
                        rcp = rcp_pool.tile([1, QB], f32, tag="rcp",
                                            name="rcp")
                        nc.vector.reciprocal_approx_fast(
                            rcp[:], st.d_ps[0:1, :])
                        b_sb = b_pool.tile([128, QB], f32, tag="bsb",
                                           name="bsb")
                        nc.gpsimd.partition_broadcast(
                            b_sb[:], rcp[0:1, :])
                        nc.vector.tensor_mul(
                            at_sb[st.h][:, st.qcol:st.qcol + QB],
                            st.o_ps[:], b_sb[:])

                def emit_proj_tile(b, tt):
                    trow = (b * NKT + tt) * 128
                    y_sb = ysb_pool.tile([128, D], bf16, tag="ysb",
                                         name="ysb")
                    for ch in range(4):
                        y_ps = y_ps_pool.tile([128, 512], f32, tag="y",
                                              name="yps")
                        for hh in range(HPC):
                            nc.tensor.matmul(
                                y_ps[:],
                                at_sb[hh][:, trow:trow + 128],
                                wo_sb[:, hh * D + ch * 512:
                                      hh * D + (ch + 1) * 512],
                                start=(hh == 0), stop=(hh == HPC - 1))
                        dst = y_sb[:, ch * 512:(ch + 1) * 512]
                        if ch % 2 == 1:
                            nc.scalar.copy(dst, y_ps[:])
                        else:
                            nc.vector.tensor_copy(dst, y_ps[:])
                    nc.sync.dma_start(
                        y_d.ap()[trow:trow + 128, :], y_sb[:])

                proj_queue = []  # (b, tt) pending projection tiles

                def drain_proj(n):
                    for _ in range(min(n, len(proj_queue))):
                        emit_proj_tile(*proj_queue.pop(0))

                for b in range(B):
                    for qb in range(NQB):
                        streams = [QbStream(b, h, qb)
                                   for h in range(HPC)]
                        ne = streams[0].ne
                        for r in range(ne):
                            for st in streams:
                                st.step()
                            if r < 3:
                                drain_proj(2)
                            elif len(proj_queue) > 2 or r % 2 == 0:
                                drain_proj(1)
                        for st in streams:
                            st.finish_pv()
                        emit_tail(streams)
                        proj_queue.extend(
                            (b, qb * 4 + t4) for t4 in range(4))
                while proj_queue:
                    drain_proj(2)
    nc.compile()
    return nc


last_results = None  # set when KERNEL_TRACE=1 (profiling from test harness)


def kernel(x, mask, W_qkv, W_o):
    import os
    import ml_dtypes
    from concourse.bass_utils import run_bass_kernel_spmd

    bf16 = ml_dtypes.bfloat16
    x = np.asarray(x, dtype=np.float32)
    mask_np = np.asarray(mask).astype(bool)
    W_qkv = np.asarray(W_qkv, dtype=np.float32)
    W_o = np.asarray(W_o, dtype=np.float32)
    m2 = np.broadcast_to(mask_np, (1, 1, S, S))[0, 0]

    key = m2.tobytes()
    nc = _cache.get(key)
    if nc is None:
        nc = _build(m2)
        _cache[key] = nc

    # x^T tiled chunk-major: xtt[c, p, kd*512+u] = x^T[kd*128+p, c*512+u]
    xt = x.reshape(T, D).T
    xtt = np.ascontiguousarray(
        xt.reshape(NDT, 128, NCH, 512).transpose(2, 1, 0, 3)
        .reshape(NCH, 128, NDT * 512).astype(bf16))

    # content-deduped mask tiles, packed [128, ntiles*QB]
    blocks = _analyze_mask(m2)
    mspecs, _ = _mask_tile_specs(m2, blocks)
    ntiles = max(1, len(mspecs))
    m2t = m2.T.astype(np.float32)  # [k, q]
    mt = np.zeros((128, ntiles * QB), np.float32)
    for i, (qb, j, m0, m1) in enumerate(mspecs):
        mt[:, i * QB:i * QB + (m1 - m0)] = \
            m2t[j * 128:(j + 1) * 128, qb * QB + m0:qb * QB + m1]

    in_maps = []
    for c in range(NCORES):
        hA, hB = HPC * c, HPC * c + 1
        q_rows = list(range(hA * DK, (hA + 1) * DK)) + \
                 list(range(hB * DK, (hB + 1) * DK))
        k_rows = [D + r for r in q_rows]
        v_rows = [2 * D + r for r in q_rows]
        wqk_c = W_qkv[q_rows + k_rows, :].T          # [D, 512]
        wqk = np.ascontiguousarray(
            wqk_c.reshape(NDT, 128, 512).transpose(1, 0, 2)
            .reshape(128, NDT * 512).astype(bf16))
        wv_c = W_qkv[v_rows, :].T                    # [D, 256]
        wv = np.ascontiguousarray(
            wv_c.reshape(NDT, 128, 256).transpose(1, 0, 2)
            .reshape(128, NDT * 256).astype(bf16))
        wo_c = W_o[:, q_rows].T                      # [256, D]
        wo = np.ascontiguousarray(
            wo_c.reshape(HPC, 128, D).transpose(1, 0, 2)
            .reshape(128, HPC * D))
        in_maps.append({"xt": xtt, "wqk": wqk, "wv": wv, "wo": wo, "mt": mt})

    trace = os.environ.get("KERNEL_TRACE", "") not in ("", "0")
    res = run_bass_kernel_spmd(nc, in_maps, core_ids=list(range(NCORES)),
                               trace=trace)
    if trace:
        global last_results
        last_results = res
    y = res.results[0]["y"].astype(np.float32)
    for c in range(1, NCORES):
        y += res.results[c]["y"].astype(np.float32)
    return y.reshape(B, S, D)
